# revision 3
# baseline (speedup 1.0000x reference)
"""TRN2 Bass kernel for AttentionRelPos (v3: token-major streaming).

Problem: B=2, T=8, S=196 (14x14), DIM=768, HEADS=12, HD=64.
  qkv = x @ qkv_w.T -> q,k,v [B, 12, 1568, 64]
  attn = softmax(q k^T / 8 + decomposed rel-pos bias)
  out = (attn @ v) heads-concat @ proj_w.T + proj_b

Sharding: 24 (batch, head) pairs -> 3 per core (8 cores). Core c handles
batch c//4, heads 3*(c%4)+[0,1,2]. Each core computes a partial final
projection over its 192 channels; the host sums the 4 partials per batch
(tensor-parallel unshard), transposes back and adds proj_b.

v3 structure (all SBUF operands fp16; PSUM fp32):
  - x streams token-major (4 blocks of 512 tokens), so each q/k projection
    psum tile completes with all 6 contraction chunks as soon as its token
    block lands: rel-pos tables and attention start ~20us earlier than the
    dim-major ordering.
  - rel bias folded into QK via augmented contraction dim 110:
    Q'[q] = [q/8, rel_h(q)|rel_t(q) @64:86, 0 @86:96, rel_w(q) @96:110],
    K'[k] = [k, onehots at matching rows].  rel_h/rel_t share one matmul
    per (t, i) via host-duplicated [64, 22] tables, and land in qt via
    direct psum->sbuf copies (partition bases 64 / 96 are 32-aligned, so
    no staging bounce).
  - exp split between Scalar (native Exp) and a custom 8-block DVE op
    computing ((a*x+b)*x+c)^16 ~ e^x (|x|<=3.5).
  - PV uses the score tile as the stationary operand: out[q,65] per
    128-q slice (65 = 64 v-dims + ones column for the denominator).
  - attnout normalized via per-partition reciprocal, transposed back to
    [c, q] with PE transposes for the projection; diag(1/D) built on
    gpsimd to keep DVE free for exp.
"""

import os
import sys

for _p in (
    "/root/.axon_site",
    "/root/.axon_site/_ro/trn_rl_repo",
    "/root/.axon_site/_ro/pypackages",
    "/opt/trn_rl_repo",
):
    if os.path.isdir(_p) and _p not in sys.path:
        sys.path.append(_p)

import numpy as np

B, T, HW_, DIM, HEADS, HD = 2, 8, 14, 768, 12, 64
S = HW_ * HW_          # 196
N = T * S              # 1568
NK = 1664              # key count padded to 13*128
KT = 13                # k tiles of 128
NF = 110               # augmented contraction: 64 q + 22 (h|t) + 10 pad + 14 w
SCALE = 0.125          # hd ** -0.5
N_CORES = 8
HPC = 3                # heads per core
QCS = (512, 512, 512, 32)   # attention q chunks
QOF = (0, 512, 1024, 1536)

# phase-1 qk rounds: (col offset, width)
ROUNDS = ((0, 256), (256, 256), (512, 256), (768, 256),
          (1024, 256), (1280, 256), (1536, 32))
XBLK = ((0, 512), (512, 512), (1024, 512), (1536, 32))

# exp(x) ~ ((EA*x + EB)*x + EC)**16, max rel err 0.70% on |x| <= 3.5
EA, EB, EC = 0.00194729, 0.06287224, 1.00006965

# const-block column offsets (fp16 columns)
C_WT = 0                      # [128, 6, 384] qk weights
C_WV = C_WT + 6 * 384         # [128, 6, 192] v weights
C_ID = C_WV + 6 * 192         # [128, 128] identity
C_RHT = C_ID + 128            # [64, 8*14*22] fused (rel_h | rel_t) tables
C_RW = C_RHT + 8 * 14 * 22    # [64, 196]
C_AUG = C_RW + 196            # [46, NK] at partitions 64:110
C_PW1 = C_AUG + NK            # [128, 768] proj rows 0:128
C_PW2 = C_PW1 + 768           # [64, 768]  proj rows 128:192
CX = C_PW2 + 768

_cached = None
_exp_op = None


def _get_exp_op():
    global _exp_op
    if _exp_op is not None:
        return _exp_op
    import concourse.dve_ops as dve_ops
    from concourse.dve_spec import Spec, Src0, C0, C1, C2, sq

    def _exp_ref(in0, in1, s0, s1, imm2):
        return ((in0 * s0 + s1) * in0 + imm2) ** 16

    op = dve_ops.DveOp(
        "EXP_POLY16_ANT",
        Spec(body=sq(sq(sq(sq((Src0 * C0 + C1) * Src0 + C2)))), reference=_exp_ref),
        subdim=False,
        uops_sha={"v3": "b9028a2770b985b4", "v4": "8a0143ec7033f2f1"},
    )
    if op.name not in dve_ops._SUB_OPCODE_FOR_NAME:
        dve_ops.OPS.append(op)
        dve_ops.CUSTOM_DVE_SPECS[op.name] = op.spec
        dve_ops._SUB_OPCODE_FOR_NAME[op.name] = (
            max(dve_ops._SUB_OPCODE_FOR_NAME.values()) + 1
        )
    _exp_op = op
    return op


def _build_bass():
    ablate = set(os.environ.get("ARP_ABLATE", "").split(","))
    import concourse.bass as bass
    import concourse.mybir as mybir
    import concourse.tile as tile
    from concourse import bacc

    exp_op = _get_exp_op()
    f32 = mybir.dt.float32
    f16 = mybir.dt.float16
    Exp = mybir.ActivationFunctionType.Exp
    Copy = mybir.ActivationFunctionType.Copy

    nc = bacc.Bacc("TRN2", target_bir_lowering=False, debug=False,
                   num_devices=N_CORES)

    d_xt = nc.dram_tensor("xt", [DIM, N], f16, kind="ExternalInput").ap()
    d_cst = nc.dram_tensor("cst", [128, CX], f16, kind="ExternalInput").ap()
    d_po = nc.dram_tensor("po", [6, 128, N], f16, kind="ExternalOutput").ap()

    with tile.TileContext(nc) as tc:
        with (
            tc.tile_pool(name="const", bufs=1) as cpool,
            tc.tile_pool(name="big", bufs=1) as bpool,
        ):
            cst = cpool.tile([128, CX], f16, tag="cst")
            xt = cpool.tile([128, 6, N], f16, tag="xt")

            # DMA order = stream priority: qk weights, x token blocks
            # interleaved with the remaining constants in need order.
            xr = d_xt[:].rearrange("(c p) n -> p c n", c=6)
            nc.sync.dma_start(cst[:, C_WT:C_WV], d_cst[:, C_WT:C_WV])
            nc.sync.dma_start(xt[:, :, 0:512], xr[:, :, 0:512])
            nc.sync.dma_start(cst[:, C_WV:C_ID], d_cst[:, C_WV:C_ID])
            nc.sync.dma_start(xt[:, :, 512:1024], xr[:, :, 512:1024])
            nc.sync.dma_start(cst[:, C_ID:C_AUG], d_cst[:, C_ID:C_AUG])
            nc.sync.dma_start(xt[:, :, 1024:1536], xr[:, :, 1024:1536])
            nc.sync.dma_start(cst[64:110, C_AUG:C_PW1],
                              d_cst[64:110, C_AUG:C_PW1])
            nc.sync.dma_start(xt[:, :, 1536:1568], xr[:, :, 1536:1568])
            nc.sync.dma_start(cst[:, C_PW1:CX], d_cst[:, C_PW1:CX])

            wt = cst[:, C_WT:C_WV].rearrange("p (c x) -> p c x", c=6)
            wv = cst[:, C_WV:C_ID].rearrange("p (c x) -> p c x", c=6)
            ident = cst[:, C_ID:C_RHT]
            rht = cst[0:64, C_RHT:C_RW]
            rwt = cst[0:64, C_RW:C_AUG]
            aug = cst[64:110, C_AUG:C_PW1]
            pw1 = cst[:, C_PW1:C_PW2]
            pw2 = cst[0:64, C_PW2:CX]

            qt = bpool.tile([NF, HPC, N], f16, tag="qt")
            kt = bpool.tile([NF, HPC, NK], f16, tag="kt")
            vp = bpool.tile([128, KT, HPC, HD + 1], f16, tag="vp")

            # zero pads: Q' rows 86:96 multiply K' zero rows (must be
            # finite; memset base must be 32-aligned, rows 64:86 are
            # overwritten by the rel copies), K' cols N:NK, PV ones column
            nc.gpsimd.memset(qt[64:96, :, :], 0.0)
            nc.gpsimd.memset(kt[0:HD, :, N:NK], 0.0)
            nc.gpsimd.memset(vp[:], 0.0)
            nc.gpsimd.memset(vp[:, 0:KT - 1, :, HD:HD + 1], 1.0)
            nc.gpsimd.memset(vp[0:32, KT - 1, :, HD:HD + 1], 1.0)

            # K' aug rows (onehots) via SBUF->SBUF DMA, head 0 first
            for h in range(HPC):
                nc.sync.dma_start(kt[HD:NF, h, :], aug)

            with (
                tc.tile_pool(name="bigp", bufs=3, space="PSUM") as bigp,
                tc.tile_pool(name="uni", bufs=2, space="PSUM") as upool,
            ):
                qt5 = qt[0:HD, :, :].rearrange(
                    "p h (t i w) -> p h t i w", t=T, i=HW_, w=HW_)
                # rel dst views: [p, t, i, h, w] and [p, t, w(=j), h, i]
                qa_ht = qt[HD:HD + 22, :, :].rearrange(
                    "p h (t i w) -> p t i h w", t=T, i=HW_, w=HW_)
                qa_w = qt[96:110, :, :].rearrange(
                    "p h (t i w) -> p t w h i", t=T, i=HW_, w=HW_)

                def emit_rel_t(t):
                    # fused rel_h|rel_t rows via [64, 22] tables, 2 halves
                    for half in range(2):
                        i0 = half * 7
                        psu = upool.tile([128, 512], f32, tag="u",
                                         name=f"rht{t}_{half}")
                        pr = psu[0:22, 0:294].rearrange(
                            "p (i c) -> p i c", i=7)
                        for i in range(i0, i0 + 7):
                            blk = C_RHT + (t * HW_ + i) * 22
                            nc.tensor.matmul(
                                pr[:, i - i0, :], cst[0:64, blk:blk + 22],
                                qt5[:, :, t, i, :], start=True, stop=True)
                        eng = (nc.scalar.copy if (t + half) % 2
                               else nc.vector.tensor_copy)
                        eng(qa_ht[:, t, i0:i0 + 7, :, :],
                            pr[:].rearrange("p i (h w) -> p i h w", h=HPC))
                    for half in range(2):
                        j0 = half * 7
                        psu = upool.tile([128, 512], f32, tag="u",
                                         name=f"rw{t}_{half}")
                        pr = psu[0:14, 0:294].rearrange(
                            "p (j c) -> p j c", j=7)
                        for j in range(j0, j0 + 7):
                            nc.tensor.matmul(
                                pr[:, j - j0, :],
                                rwt[:, j * 14:(j + 1) * 14],
                                qt5[:, :, t, :, j], start=True, stop=True)
                        eng = (nc.vector.tensor_copy if (t + half) % 2
                               else nc.scalar.copy)
                        eng(qa_w[:, t, j0:j0 + 7, :, :],
                            pr[:].rearrange("p j (h i) -> p j h i", h=HPC))

                def emit_v(nt):
                    n0 = nt * 128
                    nw = 128 if nt < KT - 1 else 32
                    psu = upool.tile([128, 512], f32, tag="u",
                                     name=f"v{nt}")
                    ps = psu[:, 0:192]
                    for c in range(6):
                        nc.tensor.matmul(ps[0:nw, :], xt[:, c, n0:n0 + nw],
                                         wv[:, c, :], start=(c == 0),
                                         stop=(c == 5))
                    dst = vp[0:nw, nt, :, 0:HD]
                    (nc.vector.tensor_copy if nt % 2 else nc.scalar.copy)(
                        dst, ps[0:nw, :].rearrange("p (h d) -> p h d", h=HPC))

                # ---- phase 1: token-major qk rounds + rel + v ----
                # rel t eligible after round covering (t+1)*196 cols;
                # v nt gated directly by its x block (emission placement
                # only -- the scheduler enforces real deps)
                rel_after = {0: [0], 1: [1], 2: [2], 3: [3, 4],
                             4: [5], 5: [6], 6: [7]}
                v_after = {1: [0, 1, 2, 3], 3: [4, 5, 6, 7],
                           5: [8, 9, 10, 11], 6: [12]}
                for r, (c0, w) in enumerate(ROUNDS):
                    bt = bigp.tile([128, 2, 512], f32, tag="big2",
                                   name=f"qk{r}")
                    pv3 = bt[:].rearrange("p a b -> p (a b)")[:, 0:3 * w]
                    pv3 = pv3.rearrange("p (m x) -> p m x", m=3)
                    for mt in range(HPC):
                        for c in range(6):
                            nc.tensor.matmul(
                                pv3[:, mt, :],
                                wt[:, c, mt * 128:(mt + 1) * 128],
                                xt[:, c, c0:c0 + w],
                                start=(c == 0), stop=(c == 5))
                    # q scaled by 1/8; alternate engines by round
                    if r % 2:
                        nc.vector.tensor_scalar_mul(
                            qt[0:HD, :, c0:c0 + w], pv3[0:HD], SCALE)
                        nc.scalar.copy(kt[0:HD, :, c0:c0 + w], pv3[HD:128])
                    else:
                        nc.scalar.activation(
                            qt[0:HD, :, c0:c0 + w], pv3[0:HD], Copy,
                            scale=SCALE)
                        nc.vector.tensor_copy(
                            kt[0:HD, :, c0:c0 + w], pv3[HD:128])
                    for t in rel_after.get(r, []):
                        if "norel" not in ablate:
                            emit_rel_t(t)
                    for nt in v_after.get(r, []):
                        emit_v(nt)

                # ---------- phase 2: attention + projection ----------
                ptp_cm = tc.tile_pool(name="ptp", bufs=5)
                aop_cm = tc.tile_pool(name="aop", bufs=4)
                aotp_cm = tc.tile_pool(name="aotp", bufs=1)
                ptp = ptp_cm.__enter__()
                aop = aop_cm.__enter__()
                aotp = aotp_cm.__enter__()
                aoT1 = aotp.tile([128, N], f16, tag="aoT1")
                aoT2 = aotp.tile([64, N], f16, tag="aoT2")
                groups = ((0, 2), (2, 2), (4, 2), (6, 2), (8, 2), (10, 2),
                          (12, 1))
                qcorder = [int(c) for c in os.environ.get("ARP_QCO", "0123")]
                units = [(qc, h) for qc in qcorder for h in range(HPC)]
                pending = []
                live = {}

                act_groups = tuple(
                    int(c) for c in os.environ.get("ARP_ACTG2", "0246"))
                diag_dve = os.environ.get("ARP_DIAGDVE", "0") == "1"

                def emit_qk_exp(i):
                    qc, h = units[i]
                    q0, qw = QOF[qc], QCS[qc]
                    ptt = ptp.tile([128, KT, 512], f16, tag="pt",
                                   name=f"pt{i}")
                    if qw <= 39:
                        # tiny tail chunk: all 13 kt fit one u slot ->
                        # one QK tile, one exp instruction per head
                        spt = upool.tile([128, 512], f32, tag="u",
                                         name=f"sp32_{i}")
                        sp32 = spt[:, 0:KT * qw].rearrange(
                            "p (k q) -> p k q", k=KT)
                        for k in range(KT):
                            nc.tensor.matmul(
                                sp32[:, k, :],
                                kt[:, h, k * 128:(k + 1) * 128],
                                qt[:, h, q0:q0 + qw],
                                start=True, stop=True,
                            )
                        if h % 2:
                            nc.scalar.activation(ptt[:, :, 0:qw], sp32[:],
                                                 Exp)
                        else:
                            nc.vector._custom_dve(
                                exp_op, out=ptt[:, :, 0:qw], in0=sp32[:],
                                s0=EA, s1=EB, imm2=EC)
                        live[i] = ptt
                        return
                    for gi in range(len(groups)):
                        g0, glen = groups[gi]
                        sp = bigp.tile([128, 2, 512], f32, tag="big2",
                                       name=f"sp2_{i}_{gi}")
                        for j in range(glen):
                            k = g0 + j
                            nc.tensor.matmul(
                                sp[:, j, 0:qw],
                                kt[:, h, k * 128:(k + 1) * 128],
                                qt[:, h, q0:q0 + qw],
                                start=True, stop=True,
                            )
                        if "noexp" in ablate:
                            (nc.scalar.copy if gi in (0, 3, 4)
                             else nc.vector.tensor_copy)(
                                ptt[:, g0:g0 + glen, 0:1],
                                sp[:, 0:glen, 0:1])
                        elif gi in act_groups:
                            nc.scalar.activation(
                                ptt[:, g0:g0 + glen, 0:qw],
                                sp[:, 0:glen, 0:qw], Exp)
                        else:
                            nc.vector._custom_dve(
                                exp_op,
                                out=ptt[:, g0:g0 + glen, 0:qw],
                                in0=sp[:, 0:glen, 0:qw],
                                s0=EA, s1=EB, imm2=EC)
                    live[i] = ptt

                def emit_pv(i):
                    qc, h = units[i]
                    q0, qw = QOF[qc], QCS[qc]
                    nsl = (qw + 127) // 128
                    ptt = live.pop(i)
                    if h == 0:
                        emit_pv.ao = aop.tile([128, 4, HPC, HD], f16,
                                              tag="ao", name=f"ao{qc}")
                        emit_pv.diag = aop.tile([128, 4, HPC, 128], f16,
                                                tag="diag", name=f"dg{qc}")
                    ao, diag = emit_pv.ao, emit_pv.diag
                    pvu = upool.tile([128, 512], f32, tag="u",
                                     name=f"pv{qc}_{h}")
                    pv = pvu[:, 0:260].rearrange("p (s d) -> p s d", s=4)
                    for s in range(nsl):
                        sw = min(128, qw - s * 128)
                        s0 = s * 128
                        for k in range(KT):
                            nc.tensor.matmul(
                                pv[0:sw, s, :],
                                ptt[:, k, s0:s0 + sw],
                                vp[:, k, h, :],
                                start=(k == 0), stop=(k == KT - 1),
                            )
                    rc = aop.tile([128, 4], f32, tag="rc", name=f"rc{qc}_{h}")
                    nc.vector.reciprocal(rc[:, 0:nsl], pv[:, 0:nsl, HD])
                    # raw (unnormalized) attnout; normalization rides the
                    # transpose matmul via a diag(1/D) moving operand
                    (nc.vector.tensor_copy if h % 2 else nc.scalar.copy)(
                        ao[:, 0:nsl, h, :], pv[:, 0:nsl, 0:HD])
                    for s in range(nsl):
                        sw = min(128, qw - s * 128)
                        if diag_dve:
                            nc.vector.tensor_scalar_mul(
                                diag[0:sw, s, h, 0:sw], ident[0:sw, 0:sw],
                                rc[0:sw, s:s + 1])
                        else:
                            nc.gpsimd.tensor_scalar_mul(
                                diag[0:sw, s, h, 0:sw], ident[0:sw, 0:sw],
                                rc[0:sw, s:s + 1])
                    if h == HPC - 1:
                        pending.append((qc, ao, diag))

                def emit_tail_a(qc, ao, diag, p0):
                    q0, qw = QOF[qc], QCS[qc]
                    nsl = (qw + 127) // 128
                    # normalize + transpose attnout back to [c, q] via PE
                    pn = min(2, nsl - p0)
                    tu = upool.tile([128, 512], f32, tag="u",
                                    name=f"t{qc}_{p0}")
                    tA = tu[:, 0:256].rearrange("p (a b) -> p a b", a=2)
                    tB = tu[0:64, 256:512].rearrange(
                        "p (a b) -> p a b", a=2)
                    for j in range(pn):
                        s = p0 + j
                        sw = min(128, qw - s * 128)
                        for h in range(HPC):
                            dst = (tA[h * 64:(h + 1) * 64, j, 0:sw]
                                   if h < 2 else tB[:, j, 0:sw])
                            nc.tensor.matmul(
                                dst, ao[0:sw, s, h, :],
                                diag[0:sw, s, h, 0:sw],
                                start=True, stop=True)
                    c0 = q0 + p0 * 128
                    cw = min(256, qw - p0 * 128)
                    dstA = aoT1[:, c0:c0 + cw]
                    dstB = aoT2[:, c0:c0 + cw]
                    if cw > 128:
                        dstA = dstA.rearrange("p (a b) -> p a b", a=2)
                        dstB = dstB.rearrange("p (a b) -> p a b", a=2)
                        srcA, srcB = tA[:, 0:2, :], tB[:, 0:2, :]
                    else:
                        dstA = dstA[:, None, :]
                        dstB = dstB[:, None, :]
                        srcA, srcB = tA[:, 0:1, 0:cw], tB[:, 0:1, 0:cw]
                    if (p0 // 2) % 2:
                        nc.vector.tensor_copy(dstA, srcA)
                        nc.scalar.copy(dstB, srcB)
                    else:
                        nc.scalar.copy(dstA, srcA)
                        nc.vector.tensor_copy(dstB, srcB)

                def emit_tail_b(qc, part):
                    q0, qw = QOF[qc], QCS[qc]
                    # partial projection for this q chunk (three 2-m slots)
                    if part == 0:
                        emit_tail_b.stg = aop.tile([128, 6, 512], f16,
                                                   tag="stg", name=f"stg{qc}")
                    stg = emit_tail_b.stg
                    for m in range(part * 2, part * 2 + 2):
                        pp = upool.tile([128, 512], f32, tag="u",
                                        name=f"pp{qc}_{m}")
                        nc.tensor.matmul(pp[:, 0:qw],
                                         pw1[:, m * 128:(m + 1) * 128],
                                         aoT1[:, q0:q0 + qw],
                                         start=True, stop=False)
                        nc.tensor.matmul(pp[:, 0:qw],
                                         pw2[:, m * 128:(m + 1) * 128],
                                         aoT2[:, q0:q0 + qw],
                                         start=False, stop=True)
                        (nc.vector.tensor_copy if m % 2 else nc.scalar.copy)(
                            stg[:, m, 0:qw], pp[:, 0:qw])
                    if part >= 1:
                        m0 = (part - 1) * 3
                        nc.sync.dma_start(
                            d_po[m0:m0 + 3, :, q0:q0 + qw].rearrange(
                                "m p q -> p m q"),
                            stg[:, m0:m0 + 3, 0:qw])

                # software pipeline: PV lags QK/exp by one unit so the PE
                # never stalls waiting for the exp of its own score tile
                jobs = []
                LAG = int(os.environ.get("ARP_LAG", "1"))
                DRIP = int(os.environ.get("ARP_DRIP", "2"))
                nu = len(units)
                for i in range(nu):
                    emit_qk_exp(i)
                    if i >= LAG:
                        emit_pv(i - LAG)
                    if pending:
                        qc0, ao0, dg0 = pending.pop(0)
                        nsl0 = (QCS[qc0] + 127) // 128
                        for p0 in range(0, nsl0, 2):
                            jobs.append((emit_tail_a, (qc0, ao0, dg0, p0)))
                        for part in range(3):
                            jobs.append((emit_tail_b, (qc0, part)))
                    for _ in range(DRIP):
                        if jobs:
                            fn, a = jobs.pop(0)
                            fn(*a)
                for i in range(nu - LAG, nu):
                    emit_pv(i)
                while jobs:
                    fn, a = jobs.pop(0)
                    fn(*a)
                while pending:
                    qc0, ao0, dg0 = pending.pop(0)
                    nsl0 = (QCS[qc0] + 127) // 128
                    for p0 in range(0, nsl0, 2):
                        emit_tail_a(qc0, ao0, dg0, p0)
                    for part in range(3):
                        emit_tail_b(qc0, part)
                aotp_cm.__exit__(None, None, None)
                aop_cm.__exit__(None, None, None)
                ptp_cm.__exit__(None, None, None)

    nc.compile()
    return nc


def _get_compiled():
    global _cached
    if _cached is None:
        _cached = _build_bass()
    return _cached


def _prepare_in_maps(x, qkv_w, proj_w, proj_b, rel_pos_h, rel_pos_w, rel_pos_t):
    f16 = np.float16
    x = np.asarray(x, np.float32)
    qkv_w = np.asarray(qkv_w, np.float32)
    proj_w = np.asarray(proj_w, np.float32)

    ii = np.arange(HW_)
    rh = 8.0 * np.asarray(rel_pos_h, np.float32)[ii[:, None] - ii[None, :] + (HW_ - 1)]
    rw = 8.0 * np.asarray(rel_pos_w, np.float32)[ii[:, None] - ii[None, :] + (HW_ - 1)]
    tt = np.arange(T)
    rt = 8.0 * np.asarray(rel_pos_t, np.float32)[tt[:, None] - tt[None, :] + (T - 1)]
    rht = rh.reshape(196, HD).T        # [64, 196]  cols: (q_h i) x (k_h)
    rwt = rw.reshape(196, HD).T        # [64, 196]
    rtt = rt.reshape(64, HD).T         # [64, 64]   cols: (q_t) x (k_t)

    # fused (rel_h | rel_t) tables: per (t, i) a [64, 22] block
    rhtdup = np.zeros((64, 8 * 14 * 22), np.float32)
    for t in range(T):
        for i in range(HW_):
            b0 = (t * HW_ + i) * 22
            rhtdup[:, b0:b0 + 14] = rht[:, i * 14:(i + 1) * 14]
            rhtdup[:, b0 + 14:b0 + 22] = rtt[:, t * 8:(t + 1) * 8]

    # K' onehot rows: 0:14 h, 14:22 t, 22:32 zero pad, 32:46 w
    aug = np.zeros((46, NK), np.float32)
    k = np.arange(N)
    aug[(k // 14) % 14, k] = 1.0
    aug[14 + k // S, k] = 1.0
    aug[32 + k % 14, k] = 1.0

    xt_b = [np.ascontiguousarray(x[b].reshape(N, DIM).T) for b in range(B)]

    in_maps = []
    for c in range(N_CORES):
        b = c // 4
        heads = [3 * (c % 4) + j for j in range(HPC)]
        wcols = []
        for h in heads:
            wcols.append(qkv_w[HD * h:HD * (h + 1), :])               # q
            wcols.append(qkv_w[DIM + HD * h:DIM + HD * (h + 1), :])   # k
        wqk = np.concatenate(wcols, axis=0).T                          # [768, 384]
        vcols = [qkv_w[2 * DIM + HD * h:2 * DIM + HD * (h + 1), :] for h in heads]
        wvv = np.concatenate(vcols, axis=0).T                          # [768, 192]
        pcols = np.concatenate([np.arange(HD * h, HD * (h + 1)) for h in heads])
        pwt = proj_w[:, pcols].T                                       # [192, 768]

        cst = np.zeros((128, CX), np.float32)
        cst[:, C_WT:C_WV] = wqk.reshape(6, 128, 384).transpose(1, 0, 2).reshape(128, -1)
        cst[:, C_WV:C_ID] = wvv.reshape(6, 128, 192).transpose(1, 0, 2).reshape(128, -1)
        cst[:, C_ID:C_RHT] = np.eye(128, dtype=np.float32)
        cst[0:64, C_RHT:C_RW] = rhtdup
        cst[0:64, C_RW:C_AUG] = rwt
        cst[64:110, C_AUG:C_PW1] = aug
        cst[:, C_PW1:C_PW2] = pwt[0:128]
        cst[0:64, C_PW2:CX] = pwt[128:192]

        in_maps.append({
            "xt": xt_b[b].astype(f16),
            "cst": cst.astype(f16),
        })
    return in_maps


def _unshard(results, proj_b, dtype):
    proj_b = np.asarray(proj_b, np.float64)
    out = np.zeros((B, T, S, DIM), dtype)
    for b in range(B):
        acc = results[4 * b]["po"].astype(np.float64)
        for c in range(4 * b + 1, 4 * b + 4):
            acc = acc + results[c]["po"].astype(np.float64)
        pot = acc.reshape(DIM, N)          # [6*128, 1568]
        out[b] = (pot.T + proj_b).reshape(T, S, DIM).astype(dtype)
    return out


def kernel(x, qkv_w, proj_w, proj_b, rel_pos_h, rel_pos_w, rel_pos_t):
    from concourse import bass_utils

    nc = _get_compiled()
    in_maps = _prepare_in_maps(x, qkv_w, proj_w, proj_b,
                               rel_pos_h, rel_pos_w, rel_pos_t)
    res = bass_utils.run_bass_kernel_spmd(nc, in_maps,
                                          core_ids=list(range(N_CORES)))
    kernel._last_results = res.results
    return _unshard(res.results, proj_b, np.asarray(x).dtype)


# revision 41
# speedup vs baseline: 1.0532x; 1.0532x over previous
"""TRN2 Bass kernel for AttentionRelPos (v3: token-major streaming).

Problem: B=2, T=8, S=196 (14x14), DIM=768, HEADS=12, HD=64.
  qkv = x @ qkv_w.T -> q,k,v [B, 12, 1568, 64]
  attn = softmax(q k^T / 8 + decomposed rel-pos bias)
  out = (attn @ v) heads-concat @ proj_w.T + proj_b

Sharding: 24 (batch, head) pairs -> 3 per core (8 cores). Core c handles
batch c//4, heads 3*(c%4)+[0,1,2]. Each core computes a partial final
projection over its 192 channels; the host sums the 4 partials per batch
(tensor-parallel unshard), transposes back and adds proj_b.

v3 structure (all SBUF operands fp16; PSUM fp32):
  - x streams token-major (4 blocks of 512 tokens), so each q/k projection
    psum tile completes with all 6 contraction chunks as soon as its token
    block lands: rel-pos tables and attention start ~20us earlier than the
    dim-major ordering.
  - rel bias folded into QK via augmented contraction dim 110:
    Q'[q] = [q/8, rel_h(q)|rel_t(q) @64:86, 0 @86:96, rel_w(q) @96:110],
    K'[k] = [k, onehots at matching rows].  rel_h/rel_t share one matmul
    per (t, i) via host-duplicated [64, 22] tables, and land in qt via
    direct psum->sbuf copies (partition bases 64 / 96 are 32-aligned, so
    no staging bounce).
  - exp split between Scalar (native Exp) and a custom 8-block DVE op
    computing ((a*x+b)*x+c)^16 ~ e^x (|x|<=3.5).
  - PV uses the score tile as the stationary operand: out[q,65] per
    128-q slice (65 = 64 v-dims + ones column for the denominator).
  - attnout normalized via per-partition reciprocal, transposed back to
    [c, q] with PE transposes for the projection; diag(1/D) built on
    gpsimd to keep DVE free for exp.
"""

import os
import sys

for _p in (
    "/root/.axon_site",
    "/root/.axon_site/_ro/trn_rl_repo",
    "/root/.axon_site/_ro/pypackages",
    "/opt/trn_rl_repo",
):
    if os.path.isdir(_p) and _p not in sys.path:
        sys.path.append(_p)

import numpy as np

B, T, HW_, DIM, HEADS, HD = 2, 8, 14, 768, 12, 64
S = HW_ * HW_          # 196
N = T * S              # 1568
NK = 1664              # key count padded to 13*128
KT = 13                # k tiles of 128
NF = 110               # augmented contraction: 64 q + 22 (h|t) + 10 pad + 14 w
SCALE = 0.125          # hd ** -0.5
N_CORES = 8
HPC = 3                # heads per core
QCS = (512, 512, 512, 32)   # attention q chunks
QOF = (0, 512, 1024, 1536)

# phase-1 x token blocks
XBLK = ((0, 512), (512, 512), (1024, 512), (1536, 32))

# exp(x) ~ ((EA*x + EB)*x + EC)**16, max rel err 0.70% on |x| <= 3.5
EA, EB, EC = 0.00194729, 0.06287224, 1.00006965

# const-block column offsets (fp16 columns)
C_WT = 0                      # [128, 6, 384] qk weights
C_WV = C_WT + 6 * 384         # [128, 6, 192] v weights
C_ID = C_WV + 6 * 192         # [128, 128] identity
C_RHT = C_ID + 128            # [64, 8*14*22] fused (rel_h | rel_t) tables
C_RW = C_RHT + 8 * 14 * 22    # [64, 196]
C_AUG = C_RW + 196            # [46, NK] at partitions 64:110
C_PW1 = C_AUG + NK            # [128, 768] proj rows 0:128
C_PW2 = C_PW1 + 768           # [64, 768]  proj rows 128:192
CX = C_PW2 + 768

_cached = None
_exp_op = None


def _get_exp_op():
    global _exp_op
    if _exp_op is not None:
        return _exp_op
    import concourse.dve_ops as dve_ops
    from concourse.dve_spec import Spec, Src0, C0, C1, C2, sq

    def _exp_ref(in0, in1, s0, s1, imm2):
        return ((in0 * s0 + s1) * in0 + imm2) ** 16

    op = dve_ops.DveOp(
        "EXP_POLY16_ANT",
        Spec(body=sq(sq(sq(sq((Src0 * C0 + C1) * Src0 + C2)))), reference=_exp_ref),
        subdim=False,
        uops_sha={"v3": "b9028a2770b985b4", "v4": "8a0143ec7033f2f1"},
    )
    if op.name not in dve_ops._SUB_OPCODE_FOR_NAME:
        dve_ops.OPS.append(op)
        dve_ops.CUSTOM_DVE_SPECS[op.name] = op.spec
        dve_ops._SUB_OPCODE_FOR_NAME[op.name] = (
            max(dve_ops._SUB_OPCODE_FOR_NAME.values()) + 1
        )
    _exp_op = op
    return op


def _build_bass():
    ablate = set(os.environ.get("ARP_ABLATE", "").split(","))
    import concourse.bass as bass
    import concourse.mybir as mybir
    import concourse.tile as tile
    from concourse import bacc

    exp_op = _get_exp_op()
    f32 = mybir.dt.float32
    f16 = mybir.dt.float16
    Exp = mybir.ActivationFunctionType.Exp
    Copy = mybir.ActivationFunctionType.Copy

    nc = bacc.Bacc("TRN2", target_bir_lowering=False, debug=False,
                   num_devices=N_CORES)

    d_xt = nc.dram_tensor("xt", [DIM, N], f16, kind="ExternalInput").ap()
    d_cst = nc.dram_tensor("cst", [128, CX], f16, kind="ExternalInput").ap()
    d_po = nc.dram_tensor("po", [6, 128, N], f16, kind="ExternalOutput").ap()

    with tile.TileContext(nc) as tc:
        with (
            tc.tile_pool(name="const", bufs=1) as cpool,
            tc.tile_pool(name="big", bufs=1) as bpool,
        ):
            cst = cpool.tile([128, CX], f16, tag="cst")
            xt = cpool.tile([128, 6, N], f16, tag="xt")

            # DMA order = stream priority: mt0 qk weights, x token blocks,
            # then remaining constants in first-use order.
            xr = d_xt[:].rearrange("(c p) n -> p c n", c=6)
            wtd = d_cst[:, C_WT:C_WV].rearrange("p (c x) -> p c x", c=6)
            wts = cst[:, C_WT:C_WV].rearrange("p (c x) -> p c x", c=6)
            nc.sync.dma_start(wts[:, :, 0:128], wtd[:, :, 0:128])
            nc.sync.dma_start(xt[:, :, 0:512], xr[:, :, 0:512])
            nc.sync.dma_start(wts[:, :, 128:384], wtd[:, :, 128:384])
            nc.sync.dma_start(xt[:, :, 512:1024], xr[:, :, 512:1024])
            nc.sync.dma_start(xt[:, :, 1024:1536], xr[:, :, 1024:1536])
            if os.environ.get("ARP_DMAORD", "1") == "1":
                nc.sync.dma_start(xt[:, :, 1536:1568], xr[:, :, 1536:1568])
                nc.sync.dma_start(cst[:, C_ID:C_AUG], d_cst[:, C_ID:C_AUG])
            else:
                nc.sync.dma_start(cst[:, C_ID:C_AUG], d_cst[:, C_ID:C_AUG])
                nc.sync.dma_start(xt[:, :, 1536:1568], xr[:, :, 1536:1568])
            nc.sync.dma_start(cst[64:110, C_AUG:C_PW1],
                              d_cst[64:110, C_AUG:C_PW1])

            wt = cst[:, C_WT:C_WV].rearrange("p (c x) -> p c x", c=6)
            wv = cst[:, C_WV:C_ID].rearrange("p (c x) -> p c x", c=6)
            ident = cst[:, C_ID:C_RHT]
            rht = cst[0:64, C_RHT:C_RW]
            rwt = cst[0:64, C_RW:C_AUG]
            aug = cst[64:110, C_AUG:C_PW1]
            pw1 = cst[:, C_PW1:C_PW2]
            pw2 = cst[0:64, C_PW2:CX]

            qt = bpool.tile([NF, HPC, N], f16, tag="qt")
            kt = bpool.tile([NF, HPC, NK], f16, tag="kt")
            vp = bpool.tile([128, KT, HPC, HD + 1], f16, tag="vp")

            # zero pads: Q' rows 86:96 multiply K' zero rows (must be
            # finite; memset base must be 32-aligned, rows 64:86 are
            # overwritten by the rel copies), K' cols N:NK, PV ones column
            junk = bpool.tile([128, 512], f16, tag="junk")
            nc.gpsimd.memset(junk[:], 0.0)
            nc.gpsimd.memset(qt[64:96, :, :], 0.0)
            nc.gpsimd.memset(kt[0:HD, :, N:NK], 0.0)
            nc.gpsimd.memset(vp[:], 0.0)
            nc.gpsimd.memset(vp[:, 0:KT - 1, :, HD:HD + 1], 1.0)
            nc.gpsimd.memset(vp[0:32, KT - 1, :, HD:HD + 1], 1.0)

            # K' aug rows (onehots) via SBUF->SBUF DMA, head 0 first
            for h in range(HPC):
                nc.sync.dma_start(kt[HD:NF, h, :], aug)
            nc.sync.dma_start(cst[:, C_WV:C_ID], d_cst[:, C_WV:C_ID])
            nc.sync.dma_start(cst[:, C_PW1:CX], d_cst[:, C_PW1:CX])

            with (
                tc.tile_pool(name="bigp", bufs=3, space="PSUM") as bigp,
                tc.tile_pool(name="uni", bufs=2, space="PSUM") as upool,
            ):
                qt5 = qt[0:HD, :, :].rearrange(
                    "p h (t i w) -> p h t i w", t=T, i=HW_, w=HW_)
                # rel dst views: [p, t, i, h, w] and [p, t, w(=j), h, i]
                qa_ht = qt[HD:HD + 22, :, :].rearrange(
                    "p h (t i w) -> p t i h w", t=T, i=HW_, w=HW_)
                qa_w = qt[96:110, :, :].rearrange(
                    "p h (t i w) -> p t w h i", t=T, i=HW_, w=HW_)

                def emit_rel_t(t):
                    # fused rel_h|rel_t rows (psum base 0) + rel_w rows
                    # (psum base 32) share one big2 slot; 2 copies
                    if "norel" in ablate:
                        return
                    bta = bigp.tile([128, 2, 512], f32, tag="big2",
                                    name=f"rel{t}")
                    # psum matmul regions must not cross the 512-f32 bank
                    # boundary: 7-index halves per bank, rel_w below rel_h|t
                    # on partitions 32:46 sharing the same columns
                    pht = bta[0:22, :, 0:294].rearrange(
                        "p g (i c) -> p g i c", i=7)
                    pw_ = bta[32:46, :, 0:294].rearrange(
                        "p g (j c) -> p g j c", j=7)
                    for i in range(HW_):
                        blk = C_RHT + (t * HW_ + i) * 22
                        nc.tensor.matmul(
                            pht[:, i // 7, i % 7, :],
                            cst[0:64, blk:blk + 22],
                            qt5[:, :, t, i, :], start=True, stop=True)
                    for j in range(HW_):
                        nc.tensor.matmul(
                            pw_[:, j // 7, j % 7, :],
                            rwt[:, j * 14:(j + 1) * 14],
                            qt5[:, :, t, :, j], start=True, stop=True)
                    eng1, eng2 = ((nc.scalar.copy, nc.vector.tensor_copy)
                                  if t % 2 else
                                  (nc.vector.tensor_copy, nc.scalar.copy))
                    eng1(qa_ht[:, t].rearrange("p (g i) h w -> p g i h w",
                                               g=2),
                         pht[:].rearrange("p g i (h w) -> p g i h w", h=HPC))
                    eng2(qa_w[:, t].rearrange("p (g j) h i -> p g j h i",
                                              g=2),
                         pw_[:].rearrange("p g j (h i) -> p g j h i", h=HPC))

                def emit_v(nt):
                    n0 = nt * 128
                    nw = 128 if nt < KT - 1 else 32
                    psu = upool.tile([128, 512], f32, tag="u",
                                     name=f"v{nt}")
                    ps = psu[:, 0:192]
                    for c in range(6):
                        nc.tensor.matmul(ps[0:nw, :], xt[:, c, n0:n0 + nw],
                                         wv[:, c, :], start=(c == 0),
                                         stop=(c == 5))
                    dst = vp[0:nw, nt, :, 0:HD]
                    (nc.vector.tensor_copy if nt % 2 else nc.scalar.copy)(
                        dst, ps[0:nw, :].rearrange("p (h d) -> p h d", h=HPC))

                # ---- phase 1: token-major qk, two (mt, qi) tiles per big2
                # slot, qi-major so each x block is consumed as it lands;
                # v owns the u slots ----
                qk_units = [(mt, qi) for qi in range(4) for mt in range(HPC)]
                first_qk = [True]
                for s in range(0, len(qk_units), 2):
                    bt = bigp.tile([128, 2, 512], f32, tag="big2",
                                   name=f"qks{s}")
                    for sub, (mt, qi) in enumerate(qk_units[s:s + 2]):
                        q0, w = QOF[qi], QCS[qi]
                        psu = bt[:, sub, :]
                        if first_qk[0]:
                            first_qk[0] = False
                            # keep PE busy through the input DMA so the
                            # p-state ramp completes before real work (junk
                            # is zeros; overwritten by the start=True c0)
                            for _ in range(int(os.environ.get("ARP_DUM", "12"))):
                                nc.tensor.matmul(psu[:, 0:512],
                                                 junk[:, 0:128],
                                                 junk[:, 0:512],
                                                 start=True, stop=False)
                        for c in range(6):
                            nc.tensor.matmul(
                                psu[:, 0:w],
                                wt[:, c, mt * 128:(mt + 1) * 128],
                                xt[:, c, q0:q0 + w],
                                start=(c == 0), stop=(c == 5))
                        if (mt + qi) % 2 == 0:
                            nc.scalar.activation(qt[0:HD, mt, q0:q0 + w],
                                                 psu[0:HD, 0:w], Copy,
                                                 scale=SCALE)
                            nc.vector.tensor_copy(kt[0:HD, mt, q0:q0 + w],
                                                  psu[HD:128, 0:w])
                        else:
                            nc.vector.tensor_scalar_mul(
                                qt[0:HD, mt, q0:q0 + w], psu[0:HD, 0:w],
                                SCALE)
                            nc.scalar.copy(kt[0:HD, mt, q0:q0 + w],
                                           psu[HD:128, 0:w])
                for t in (0, 1, 2):
                    emit_rel_t(t)
                for nt in range(KT):
                    emit_v(nt)

                # ---------- phase 2: attention + projection ----------
                ptp_cm = tc.tile_pool(name="ptp", bufs=int(os.environ.get("ARP_PTP", "5")))
                aop_cm = tc.tile_pool(name="aop", bufs=4)
                aotp_cm = tc.tile_pool(name="aotp", bufs=1)
                ptp = ptp_cm.__enter__()
                aop = aop_cm.__enter__()
                aotp = aotp_cm.__enter__()
                aoT1 = aotp.tile([128, N], f16, tag="aoT1")
                aoT2 = aotp.tile([64, N], f16, tag="aoT2")
                groups = ((0, 2), (2, 2), (4, 2), (6, 2), (8, 2), (10, 2),
                          (12, 1))
                qcorder = [int(c) for c in os.environ.get("ARP_QCO", "0123")]
                units = [(qc, h) for qc in qcorder for h in range(HPC)]
                pending = []
                live = {}

                act_groups = tuple(
                    int(c) for c in os.environ.get("ARP_ACTG2", "0246"))
                diag_dve = os.environ.get("ARP_DIAGDVE", "0") == "1"

                def emit_qk_exp(i):
                    qc, h = units[i]
                    q0, qw = QOF[qc], QCS[qc]
                    ptt = ptp.tile([128, KT, 512], f16, tag="pt",
                                   name=f"pt{i}")
                    if qw <= 39:
                        # tiny tail chunk: all 13 kt fit one big2 slot ->
                        # one QK tile, one exp instruction per head
                        spt = bigp.tile([128, 2, 512], f32, tag="big2",
                                        name=f"sp32_{i}")
                        sp32 = spt[:].rearrange("p a b -> p (a b)")[
                            :, 0:KT * qw].rearrange("p (k q) -> p k q", k=KT)
                        for k in range(KT):
                            nc.tensor.matmul(
                                sp32[:, k, :],
                                kt[:, h, k * 128:(k + 1) * 128],
                                qt[:, h, q0:q0 + qw],
                                start=True, stop=True,
                            )
                        if h % 2:
                            nc.scalar.activation(ptt[:, :, 0:qw], sp32[:],
                                                 Exp)
                        else:
                            nc.vector._custom_dve(
                                exp_op, out=ptt[:, :, 0:qw], in0=sp32[:],
                                s0=EA, s1=EB, imm2=EC)
                        live[i] = ptt
                        return
                    for gi in range(len(groups)):
                        g0, glen = groups[gi]
                        sp = bigp.tile([128, 2, 512], f32, tag="big2",
                                       name=f"sp2_{i}_{gi}")
                        for j in range(glen):
                            k = g0 + j
                            nc.tensor.matmul(
                                sp[:, j, 0:qw],
                                kt[:, h, k * 128:(k + 1) * 128],
                                qt[:, h, q0:q0 + qw],
                                start=True, stop=True,
                            )
                        if "noexp" in ablate:
                            (nc.scalar.copy if gi in (0, 3, 4)
                             else nc.vector.tensor_copy)(
                                ptt[:, g0:g0 + glen, 0:1],
                                sp[:, 0:glen, 0:1])
                        elif gi in act_groups:
                            nc.scalar.activation(
                                ptt[:, g0:g0 + glen, 0:qw],
                                sp[:, 0:glen, 0:qw], Exp)
                        else:
                            nc.vector._custom_dve(
                                exp_op,
                                out=ptt[:, g0:g0 + glen, 0:qw],
                                in0=sp[:, 0:glen, 0:qw],
                                s0=EA, s1=EB, imm2=EC)
                    live[i] = ptt

                def emit_pv(i):
                    qc, h = units[i]
                    q0, qw = QOF[qc], QCS[qc]
                    nsl = (qw + 127) // 128
                    ptt = live.pop(i)
                    if h == 0:
                        emit_pv.ao = aop.tile([128, 4, HPC, HD], f16,
                                              tag="ao", name=f"ao{qc}")
                        emit_pv.diag = aop.tile([128, 4, HPC, 128], f16,
                                                tag="diag", name=f"dg{qc}")
                    ao, diag = emit_pv.ao, emit_pv.diag
                    pvu = upool.tile([128, 512], f32, tag="u",
                                     name=f"pv{qc}_{h}")
                    pv = pvu[:, 0:260].rearrange("p (s d) -> p s d", s=4)
                    for s in range(nsl):
                        sw = min(128, qw - s * 128)
                        s0 = s * 128
                        for k in range(KT):
                            nc.tensor.matmul(
                                pv[0:sw, s, :],
                                ptt[:, k, s0:s0 + sw],
                                vp[:, k, h, :],
                                start=(k == 0), stop=(k == KT - 1),
                            )
                    rc = aop.tile([128, 4], f32, tag="rc", name=f"rc{qc}_{h}")
                    nc.vector.reciprocal(rc[:, 0:nsl], pv[:, 0:nsl, HD])
                    # raw (unnormalized) attnout; normalization rides the
                    # transpose matmul via a diag(1/D) moving operand
                    (nc.vector.tensor_copy if h % 2 else nc.scalar.copy)(
                        ao[:, 0:nsl, h, :], pv[:, 0:nsl, 0:HD])
                    for s in range(nsl):
                        sw = min(128, qw - s * 128)
                        if diag_dve:
                            nc.vector.tensor_scalar_mul(
                                diag[0:sw, s, h, 0:sw], ident[0:sw, 0:sw],
                                rc[0:sw, s:s + 1])
                        else:
                            nc.gpsimd.tensor_scalar_mul(
                                diag[0:sw, s, h, 0:sw], ident[0:sw, 0:sw],
                                rc[0:sw, s:s + 1])
                    if h == HPC - 1:
                        pending.append((qc, ao, diag))

                def emit_tail_a(qc, ao, diag, p0):
                    q0, qw = QOF[qc], QCS[qc]
                    nsl = (qw + 127) // 128
                    # normalize + transpose attnout back to [c, q] via PE
                    pn = min(2, nsl - p0)
                    tu = upool.tile([128, 512], f32, tag="u",
                                    name=f"t{qc}_{p0}")
                    tA = tu[:, 0:256].rearrange("p (a b) -> p a b", a=2)
                    tB = tu[0:64, 256:512].rearrange(
                        "p (a b) -> p a b", a=2)
                    for j in range(pn):
                        s = p0 + j
                        sw = min(128, qw - s * 128)
                        for h in range(HPC):
                            dst = (tA[h * 64:(h + 1) * 64, j, 0:sw]
                                   if h < 2 else tB[:, j, 0:sw])
                            nc.tensor.matmul(
                                dst, ao[0:sw, s, h, :],
                                diag[0:sw, s, h, 0:sw],
                                start=True, stop=True)
                    c0 = q0 + p0 * 128
                    cw = min(256, qw - p0 * 128)
                    dstA = aoT1[:, c0:c0 + cw]
                    dstB = aoT2[:, c0:c0 + cw]
                    if cw > 128:
                        dstA = dstA.rearrange("p (a b) -> p a b", a=2)
                        dstB = dstB.rearrange("p (a b) -> p a b", a=2)
                        srcA, srcB = tA[:, 0:2, :], tB[:, 0:2, :]
                    else:
                        dstA = dstA[:, None, :]
                        dstB = dstB[:, None, :]
                        srcA, srcB = tA[:, 0:1, 0:cw], tB[:, 0:1, 0:cw]
                    if (p0 // 2) % 2:
                        nc.vector.tensor_copy(dstA, srcA)
                        nc.scalar.copy(dstB, srcB)
                    else:
                        nc.scalar.copy(dstA, srcA)
                        nc.vector.tensor_copy(dstB, srcB)

                def emit_tail_b(qc, part):
                    q0, qw = QOF[qc], QCS[qc]
                    # partial projection for this q chunk (three 2-m slots)
                    if part == 0:
                        emit_tail_b.stg = aop.tile([128, 6, 512], f16,
                                                   tag="stg", name=f"stg{qc}")
                    stg = emit_tail_b.stg
                    ppt = bigp.tile([128, 2, 512], f32, tag="big2",
                                    name=f"pp{qc}_{part}")
                    for m in range(part * 2, part * 2 + 2):
                        pp = ppt[:, m % 2, :]
                        nc.tensor.matmul(pp[:, 0:qw],
                                         pw1[:, m * 128:(m + 1) * 128],
                                         aoT1[:, q0:q0 + qw],
                                         start=True, stop=False)
                        nc.tensor.matmul(pp[:, 0:qw],
                                         pw2[:, m * 128:(m + 1) * 128],
                                         aoT2[:, q0:q0 + qw],
                                         start=False, stop=True)
                        (nc.vector.tensor_copy if m % 2 else nc.scalar.copy)(
                            stg[:, m, 0:qw], pp[:, 0:qw])
                    if part >= 1:
                        m0 = (part - 1) * 3
                        nc.sync.dma_start(
                            d_po[m0:m0 + 3, :, q0:q0 + qw].rearrange(
                                "m p q -> p m q"),
                            stg[:, m0:m0 + 3, 0:qw])

                # software pipeline: PV lags QK/exp by one unit so the PE
                # never stalls waiting for the exp of its own score tile
                jobs = []
                LAG = int(os.environ.get("ARP_LAG", "2"))
                DRIP = int(os.environ.get("ARP_DRIP", "1"))
                nu = len(units)
                for i in range(nu):
                    emit_qk_exp(i)
                    if i == 0:
                        for t in (3, 4):
                            emit_rel_t(t)
                    elif i == 1:
                        for t in (5, 6, 7):
                            emit_rel_t(t)
                    if i >= LAG:
                        emit_pv(i - LAG)
                    if pending:
                        qc0, ao0, dg0 = pending.pop(0)
                        nsl0 = (QCS[qc0] + 127) // 128
                        for p0 in range(0, nsl0, 2):
                            jobs.append((emit_tail_a, (qc0, ao0, dg0, p0)))
                        for part in range(3):
                            jobs.append((emit_tail_b, (qc0, part)))
                    for _ in range(DRIP):
                        if jobs:
                            fn, a = jobs.pop(0)
                            fn(*a)
                for i in range(nu - LAG, nu):
                    emit_pv(i)
                while jobs:
                    fn, a = jobs.pop(0)
                    fn(*a)
                while pending:
                    qc0, ao0, dg0 = pending.pop(0)
                    nsl0 = (QCS[qc0] + 127) // 128
                    for p0 in range(0, nsl0, 2):
                        emit_tail_a(qc0, ao0, dg0, p0)
                    for part in range(3):
                        emit_tail_b(qc0, part)
                aotp_cm.__exit__(None, None, None)
                aop_cm.__exit__(None, None, None)
                ptp_cm.__exit__(None, None, None)

    nc.compile()
    return nc


def _get_compiled():
    global _cached
    if _cached is None:
        _cached = _build_bass()
    return _cached


def _prepare_in_maps(x, qkv_w, proj_w, proj_b, rel_pos_h, rel_pos_w, rel_pos_t):
    f16 = np.float16
    x = np.asarray(x, np.float32)
    qkv_w = np.asarray(qkv_w, np.float32)
    proj_w = np.asarray(proj_w, np.float32)

    ii = np.arange(HW_)
    rh = 8.0 * np.asarray(rel_pos_h, np.float32)[ii[:, None] - ii[None, :] + (HW_ - 1)]
    rw = 8.0 * np.asarray(rel_pos_w, np.float32)[ii[:, None] - ii[None, :] + (HW_ - 1)]
    tt = np.arange(T)
    rt = 8.0 * np.asarray(rel_pos_t, np.float32)[tt[:, None] - tt[None, :] + (T - 1)]
    rht = rh.reshape(196, HD).T        # [64, 196]  cols: (q_h i) x (k_h)
    rwt = rw.reshape(196, HD).T        # [64, 196]
    rtt = rt.reshape(64, HD).T         # [64, 64]   cols: (q_t) x (k_t)

    # fused (rel_h | rel_t) tables: per (t, i) a [64, 22] block
    rhtdup = np.zeros((64, 8 * 14 * 22), np.float32)
    for t in range(T):
        for i in range(HW_):
            b0 = (t * HW_ + i) * 22
            rhtdup[:, b0:b0 + 14] = rht[:, i * 14:(i + 1) * 14]
            rhtdup[:, b0 + 14:b0 + 22] = rtt[:, t * 8:(t + 1) * 8]

    # K' onehot rows: 0:14 h, 14:22 t, 22:32 zero pad, 32:46 w
    aug = np.zeros((46, NK), np.float32)
    k = np.arange(N)
    aug[(k // 14) % 14, k] = 1.0
    aug[14 + k // S, k] = 1.0
    aug[32 + k % 14, k] = 1.0

    xt_b = [np.ascontiguousarray(x[b].reshape(N, DIM).T) for b in range(B)]

    in_maps = []
    for c in range(N_CORES):
        b = c // 4
        heads = [3 * (c % 4) + j for j in range(HPC)]
        wcols = []
        for h in heads:
            wcols.append(qkv_w[HD * h:HD * (h + 1), :])               # q
            wcols.append(qkv_w[DIM + HD * h:DIM + HD * (h + 1), :])   # k
        wqk = np.concatenate(wcols, axis=0).T                          # [768, 384]
        vcols = [qkv_w[2 * DIM + HD * h:2 * DIM + HD * (h + 1), :] for h in heads]
        wvv = np.concatenate(vcols, axis=0).T                          # [768, 192]
        pcols = np.concatenate([np.arange(HD * h, HD * (h + 1)) for h in heads])
        pwt = proj_w[:, pcols].T                                       # [192, 768]

        cst = np.zeros((128, CX), np.float32)
        cst[:, C_WT:C_WV] = wqk.reshape(6, 128, 384).transpose(1, 0, 2).reshape(128, -1)
        cst[:, C_WV:C_ID] = wvv.reshape(6, 128, 192).transpose(1, 0, 2).reshape(128, -1)
        cst[:, C_ID:C_RHT] = np.eye(128, dtype=np.float32)
        cst[0:64, C_RHT:C_RW] = rhtdup
        cst[0:64, C_RW:C_AUG] = rwt
        cst[64:110, C_AUG:C_PW1] = aug
        cst[:, C_PW1:C_PW2] = pwt[0:128]
        cst[0:64, C_PW2:CX] = pwt[128:192]

        in_maps.append({
            "xt": xt_b[b].astype(f16),
            "cst": cst.astype(f16),
        })
    return in_maps


def _unshard(results, proj_b, dtype):
    proj_b = np.asarray(proj_b, np.float64)
    out = np.zeros((B, T, S, DIM), dtype)
    for b in range(B):
        acc = results[4 * b]["po"].astype(np.float64)
        for c in range(4 * b + 1, 4 * b + 4):
            acc = acc + results[c]["po"].astype(np.float64)
        pot = acc.reshape(DIM, N)          # [6*128, 1568]
        out[b] = (pot.T + proj_b).reshape(T, S, DIM).astype(dtype)
    return out


def kernel(x, qkv_w, proj_w, proj_b, rel_pos_h, rel_pos_w, rel_pos_t):
    from concourse import bass_utils

    nc = _get_compiled()
    in_maps = _prepare_in_maps(x, qkv_w, proj_w, proj_b,
                               rel_pos_h, rel_pos_w, rel_pos_t)
    res = bass_utils.run_bass_kernel_spmd(nc, in_maps,
                                          core_ids=list(range(N_CORES)))
    kernel._last_results = res.results
    return _unshard(res.results, proj_b, np.asarray(x).dtype)


# revision 44
# speedup vs baseline: 1.0936x; 1.0384x over previous
"""TRN2 Bass kernel for AttentionRelPos (v3: token-major streaming).

Problem: B=2, T=8, S=196 (14x14), DIM=768, HEADS=12, HD=64.
  qkv = x @ qkv_w.T -> q,k,v [B, 12, 1568, 64]
  attn = softmax(q k^T / 8 + decomposed rel-pos bias)
  out = (attn @ v) heads-concat @ proj_w.T + proj_b

Sharding: 24 (batch, head) pairs -> 3 per core (8 cores). Core c handles
batch c//4, heads 3*(c%4)+[0,1,2]. Each core computes a partial final
projection over its 192 channels; the host sums the 4 partials per batch
(tensor-parallel unshard), transposes back and adds proj_b.

v3 structure (all SBUF operands fp16; PSUM fp32):
  - x streams token-major (4 blocks of 512 tokens), so each q/k projection
    psum tile completes with all 6 contraction chunks as soon as its token
    block lands: rel-pos tables and attention start ~20us earlier than the
    dim-major ordering.
  - rel bias folded into QK via augmented contraction dim 110:
    Q'[q] = [q/8, rel_h(q)|rel_t(q) @64:86, 0 @86:96, rel_w(q) @96:110],
    K'[k] = [k, onehots at matching rows].  rel_h/rel_t share one matmul
    per (t, i) via host-duplicated [64, 22] tables, and land in qt via
    direct psum->sbuf copies (partition bases 64 / 96 are 32-aligned, so
    no staging bounce).
  - exp split between Scalar (native Exp) and a custom 8-block DVE op
    computing ((a*x+b)*x+c)^16 ~ e^x (|x|<=3.5).
  - PV uses the score tile as the stationary operand: out[q,65] per
    128-q slice (65 = 64 v-dims + ones column for the denominator).
  - attnout normalized via per-partition reciprocal, transposed back to
    [c, q] with PE transposes for the projection; diag(1/D) built on
    gpsimd to keep DVE free for exp.
"""

import os
import sys

for _p in (
    "/root/.axon_site",
    "/root/.axon_site/_ro/trn_rl_repo",
    "/root/.axon_site/_ro/pypackages",
    "/opt/trn_rl_repo",
):
    if os.path.isdir(_p) and _p not in sys.path:
        sys.path.append(_p)

import numpy as np

B, T, HW_, DIM, HEADS, HD = 2, 8, 14, 768, 12, 64
S = HW_ * HW_          # 196
N = T * S              # 1568
NK = 1664              # key count padded to 13*128
KT = 13                # k tiles of 128
NF = 110               # augmented contraction: 64 q + 22 (h|t) + 10 pad + 14 w
SCALE = 0.125          # hd ** -0.5
N_CORES = 8
HPC = 3                # heads per core
QCS = (512, 512, 512, 32)   # attention q chunks
QOF = (0, 512, 1024, 1536)

# phase-1 x token blocks
XBLK = ((0, 512), (512, 512), (1024, 512), (1536, 32))

# exp(x) ~ ((EA*x + EB)*x + EC)**16, max rel err 0.70% on |x| <= 3.5
EA, EB, EC = 0.00194729, 0.06287224, 1.00006965

# const-block column offsets (fp16 columns)
C_WT = 0                      # [128, 6, 384] qk weights
C_WV = C_WT + 6 * 384         # [128, 6, 192] v weights
C_ID = C_WV + 6 * 192         # [128, 128] identity
C_RHT = C_ID + 128            # [64, 8*14*22] fused (rel_h | rel_t) tables
C_RW = C_RHT + 8 * 14 * 22    # [64, 196]
C_AUG = C_RW + 196            # [46, NK] at partitions 64:110
C_PW1 = C_AUG + NK            # [128, 768] proj rows 0:128
C_PW2 = C_PW1 + 768           # [64, 768]  proj rows 128:192
CX = C_PW2 + 768

_cached = None
_exp_op = None


def _get_exp_op():
    global _exp_op
    if _exp_op is not None:
        return _exp_op
    import concourse.dve_ops as dve_ops
    from concourse.dve_spec import Spec, Src0, C0, C1, C2, sq

    def _exp_ref(in0, in1, s0, s1, imm2):
        return ((in0 * s0 + s1) * in0 + imm2) ** 16

    op = dve_ops.DveOp(
        "EXP_POLY16_ANT",
        Spec(body=sq(sq(sq(sq((Src0 * C0 + C1) * Src0 + C2)))), reference=_exp_ref),
        subdim=False,
        uops_sha={"v3": "b9028a2770b985b4", "v4": "8a0143ec7033f2f1"},
    )
    if op.name not in dve_ops._SUB_OPCODE_FOR_NAME:
        dve_ops.OPS.append(op)
        dve_ops.CUSTOM_DVE_SPECS[op.name] = op.spec
        dve_ops._SUB_OPCODE_FOR_NAME[op.name] = (
            max(dve_ops._SUB_OPCODE_FOR_NAME.values()) + 1
        )
    _exp_op = op
    return op


def _build_bass():
    ablate = set(os.environ.get("ARP_ABLATE", "").split(","))
    import concourse.bass as bass
    import concourse.mybir as mybir
    import concourse.tile as tile
    from concourse import bacc

    exp_op = _get_exp_op()
    f32 = mybir.dt.float32
    f16 = mybir.dt.float16
    Exp = mybir.ActivationFunctionType.Exp
    Copy = mybir.ActivationFunctionType.Copy

    nc = bacc.Bacc("TRN2", target_bir_lowering=False, debug=False,
                   num_devices=N_CORES)

    d_xt = nc.dram_tensor("xt", [DIM, N], f16, kind="ExternalInput").ap()
    d_cst = nc.dram_tensor("cst", [128, CX], f16, kind="ExternalInput").ap()
    d_po = nc.dram_tensor("po", [6, 128, N], f16, kind="ExternalOutput").ap()

    with tile.TileContext(nc) as tc:
        with (
            tc.tile_pool(name="const", bufs=1) as cpool,
            tc.tile_pool(name="big", bufs=1) as bpool,
        ):
            cst = cpool.tile([128, CX], f16, tag="cst")
            xt = cpool.tile([128, 6, N], f16, tag="xt")

            # DMA order = stream priority: mt0 qk weights, x token blocks,
            # then remaining constants in first-use order.
            xr = d_xt[:].rearrange("(c p) n -> p c n", c=6)
            wtd = d_cst[:, C_WT:C_WV].rearrange("p (c x) -> p c x", c=6)
            wts = cst[:, C_WT:C_WV].rearrange("p (c x) -> p c x", c=6)
            nc.sync.dma_start(wts[:, :, 0:128], wtd[:, :, 0:128])
            nc.sync.dma_start(xt[:, :, 0:512], xr[:, :, 0:512])
            nc.sync.dma_start(wts[:, :, 128:384], wtd[:, :, 128:384])
            nc.sync.dma_start(xt[:, :, 512:1024], xr[:, :, 512:1024])
            nc.sync.dma_start(xt[:, :, 1024:1536], xr[:, :, 1024:1536])
            if os.environ.get("ARP_DMAORD", "1") == "1":
                nc.sync.dma_start(xt[:, :, 1536:1568], xr[:, :, 1536:1568])
                nc.sync.dma_start(cst[:, C_ID:C_AUG], d_cst[:, C_ID:C_AUG])
            else:
                nc.sync.dma_start(cst[:, C_ID:C_AUG], d_cst[:, C_ID:C_AUG])
                nc.sync.dma_start(xt[:, :, 1536:1568], xr[:, :, 1536:1568])
            nc.sync.dma_start(cst[64:110, C_AUG:C_PW1],
                              d_cst[64:110, C_AUG:C_PW1])

            wt = cst[:, C_WT:C_WV].rearrange("p (c x) -> p c x", c=6)
            wv = cst[:, C_WV:C_ID].rearrange("p (c x) -> p c x", c=6)
            ident = cst[:, C_ID:C_RHT]
            rht = cst[0:64, C_RHT:C_RW]
            rwt = cst[0:64, C_RW:C_AUG]
            aug = cst[64:110, C_AUG:C_PW1]
            pw1 = cst[:, C_PW1:C_PW2]
            pw2 = cst[0:64, C_PW2:CX]

            qt = bpool.tile([NF, HPC, N], f16, tag="qt")
            kt = bpool.tile([NF, HPC, NK], f16, tag="kt")
            vp = bpool.tile([128, KT, HPC, HD + 1], f16, tag="vp")

            # zero pads: Q' rows 86:96 multiply K' zero rows (must be
            # finite; memset base must be 32-aligned, rows 64:86 are
            # overwritten by the rel copies), K' cols N:NK, PV ones column
            junk = bpool.tile([128, 512], f16, tag="junk")
            nc.gpsimd.memset(junk[:], 0.0)
            nc.gpsimd.memset(qt[64:96, :, :], 0.0)
            nc.gpsimd.memset(kt[0:HD, :, N:NK], 0.0)
            nc.gpsimd.memset(vp[:], 0.0)
            nc.gpsimd.memset(vp[:, 0:KT - 1, :, HD:HD + 1], 1.0)
            nc.gpsimd.memset(vp[0:32, KT - 1, :, HD:HD + 1], 1.0)

            # K' aug rows (onehots) via SBUF->SBUF DMA, head 0 first
            for h in range(HPC):
                nc.sync.dma_start(kt[HD:NF, h, :], aug)
            nc.sync.dma_start(cst[:, C_WV:C_ID], d_cst[:, C_WV:C_ID])
            nc.sync.dma_start(cst[:, C_PW1:CX], d_cst[:, C_PW1:CX])

            with (
                tc.tile_pool(name="bigp", bufs=3, space="PSUM") as bigp,
                tc.tile_pool(name="uni", bufs=2, space="PSUM") as upool,
            ):
                qt5 = qt[0:HD, :, :].rearrange(
                    "p h (t i w) -> p h t i w", t=T, i=HW_, w=HW_)
                # rel dst views: [p, t, i, h, w] and [p, t, w(=j), h, i]
                qa_ht = qt[HD:HD + 22, :, :].rearrange(
                    "p h (t i w) -> p t i h w", t=T, i=HW_, w=HW_)
                qa_w = qt[96:110, :, :].rearrange(
                    "p h (t i w) -> p t w h i", t=T, i=HW_, w=HW_)

                def emit_rel_t(t):
                    # fused rel_h|rel_t rows (psum base 0) + rel_w rows
                    # (psum base 32) share one big2 slot; 2 copies
                    if "norel" in ablate:
                        return
                    bta = bigp.tile([128, 2, 512], f32, tag="big2",
                                    name=f"rel{t}")
                    # psum matmul regions must not cross the 512-f32 bank
                    # boundary: 7-index halves per bank, rel_w below rel_h|t
                    # on partitions 32:46 sharing the same columns
                    pht = bta[0:22, :, 0:294].rearrange(
                        "p g (i c) -> p g i c", i=7)
                    pw_ = bta[32:46, :, 0:294].rearrange(
                        "p g (j c) -> p g j c", j=7)
                    for i in range(HW_):
                        blk = C_RHT + (t * HW_ + i) * 22
                        nc.tensor.matmul(
                            pht[:, i // 7, i % 7, :],
                            cst[0:64, blk:blk + 22],
                            qt5[:, :, t, i, :], start=True, stop=True)
                    for j in range(HW_):
                        nc.tensor.matmul(
                            pw_[:, j // 7, j % 7, :],
                            rwt[:, j * 14:(j + 1) * 14],
                            qt5[:, :, t, :, j], start=True, stop=True)
                    eng1, eng2 = ((nc.scalar.copy, nc.vector.tensor_copy)
                                  if t % 2 else
                                  (nc.vector.tensor_copy, nc.scalar.copy))
                    eng1(qa_ht[:, t].rearrange("p (g i) h w -> p g i h w",
                                               g=2),
                         pht[:].rearrange("p g i (h w) -> p g i h w", h=HPC))
                    eng2(qa_w[:, t].rearrange("p (g j) h i -> p g j h i",
                                              g=2),
                         pw_[:].rearrange("p g j (h i) -> p g j h i", h=HPC))

                def emit_v(nt):
                    n0 = nt * 128
                    nw = 128 if nt < KT - 1 else 32
                    psu = upool.tile([128, 512], f32, tag="u",
                                     name=f"v{nt}")
                    ps = psu[:, 0:192]
                    for c in range(6):
                        nc.tensor.matmul(ps[0:nw, :], xt[:, c, n0:n0 + nw],
                                         wv[:, c, :], start=(c == 0),
                                         stop=(c == 5))
                    dst = vp[0:nw, nt, :, 0:HD]
                    (nc.vector.tensor_copy if nt % 2 else nc.scalar.copy)(
                        dst, ps[0:nw, :].rearrange("p (h d) -> p h d", h=HPC))

                # ---- phase 1: token-major qk, two (mt, qi) tiles per big2
                # slot, qi-major so each x block is consumed as it lands;
                # v owns the u slots ----
                qk_units = [(mt, qi) for qi in range(4) for mt in range(HPC)]
                first_qk = [True]
                for s in range(0, len(qk_units), 2):
                    bt = bigp.tile([128, 2, 512], f32, tag="big2",
                                   name=f"qks{s}")
                    for sub, (mt, qi) in enumerate(qk_units[s:s + 2]):
                        q0, w = QOF[qi], QCS[qi]
                        psu = bt[:, sub, :]
                        if first_qk[0]:
                            first_qk[0] = False
                            # keep PE busy through the input DMA so the
                            # p-state ramp completes before real work (junk
                            # is zeros; overwritten by the start=True c0)
                            for _ in range(int(os.environ.get("ARP_DUM", "12"))):
                                nc.tensor.matmul(psu[:, 0:512],
                                                 junk[:, 0:128],
                                                 junk[:, 0:512],
                                                 start=True, stop=False)
                        for c in range(6):
                            nc.tensor.matmul(
                                psu[:, 0:w],
                                wt[:, c, mt * 128:(mt + 1) * 128],
                                xt[:, c, q0:q0 + w],
                                start=(c == 0), stop=(c == 5))
                        if (mt + qi) % 2 == 0:
                            nc.scalar.activation(qt[0:HD, mt, q0:q0 + w],
                                                 psu[0:HD, 0:w], Copy,
                                                 scale=SCALE)
                            nc.vector.tensor_copy(kt[0:HD, mt, q0:q0 + w],
                                                  psu[HD:128, 0:w])
                        else:
                            nc.vector.tensor_scalar_mul(
                                qt[0:HD, mt, q0:q0 + w], psu[0:HD, 0:w],
                                SCALE)
                            nc.scalar.copy(kt[0:HD, mt, q0:q0 + w],
                                           psu[HD:128, 0:w])
                relinj = os.environ.get("ARP_RELINJ", "pre")
                for t in (0, 1, 2):
                    emit_rel_t(t)
                if relinj == "pre":
                    for t in (3, 4, 5, 6, 7):
                        emit_rel_t(t)
                for nt in range(KT):
                    emit_v(nt)
                if relinj == "postv":
                    for t in (3, 4, 5, 6, 7):
                        emit_rel_t(t)

                # ---------- phase 2: attention + projection ----------
                ptp_cm = tc.tile_pool(name="ptp", bufs=int(os.environ.get("ARP_PTP", "5")))
                aop_cm = tc.tile_pool(name="aop", bufs=4)
                aotp_cm = tc.tile_pool(name="aotp", bufs=1)
                ptp = ptp_cm.__enter__()
                aop = aop_cm.__enter__()
                aotp = aotp_cm.__enter__()
                aoT1 = aotp.tile([128, N], f16, tag="aoT1")
                aoT2 = aotp.tile([64, N], f16, tag="aoT2")
                groups = ((0, 2), (2, 2), (4, 2), (6, 2), (8, 2), (10, 2),
                          (12, 1))
                qcorder = [int(c) for c in os.environ.get("ARP_QCO", "0123")]
                units = [(qc, h) for qc in qcorder for h in range(HPC)]
                pending = []
                live = {}

                act_groups = tuple(
                    int(c) for c in os.environ.get("ARP_ACTG2", "0246"))
                diag_dve = os.environ.get("ARP_DIAGDVE", "1") == "1"

                def emit_qk_exp(i):
                    qc, h = units[i]
                    q0, qw = QOF[qc], QCS[qc]
                    ptt = ptp.tile([128, KT, 512], f16, tag="pt",
                                   name=f"pt{i}")
                    if qw <= 39:
                        # tiny tail chunk: all 13 kt fit one big2 slot ->
                        # one QK tile, one exp instruction per head
                        spt = bigp.tile([128, 2, 512], f32, tag="big2",
                                        name=f"sp32_{i}")
                        sp32 = spt[:].rearrange("p a b -> p (a b)")[
                            :, 0:KT * qw].rearrange("p (k q) -> p k q", k=KT)
                        for k in range(KT):
                            nc.tensor.matmul(
                                sp32[:, k, :],
                                kt[:, h, k * 128:(k + 1) * 128],
                                qt[:, h, q0:q0 + qw],
                                start=True, stop=True,
                            )
                        if h % 2:
                            nc.scalar.activation(ptt[:, :, 0:qw], sp32[:],
                                                 Exp)
                        else:
                            nc.vector._custom_dve(
                                exp_op, out=ptt[:, :, 0:qw], in0=sp32[:],
                                s0=EA, s1=EB, imm2=EC)
                        live[i] = ptt
                        return
                    for gi in range(len(groups)):
                        g0, glen = groups[gi]
                        sp = bigp.tile([128, 2, 512], f32, tag="big2",
                                       name=f"sp2_{i}_{gi}")
                        for j in range(glen):
                            k = g0 + j
                            nc.tensor.matmul(
                                sp[:, j, 0:qw],
                                kt[:, h, k * 128:(k + 1) * 128],
                                qt[:, h, q0:q0 + qw],
                                start=True, stop=True,
                            )
                        if "noexp" in ablate:
                            (nc.scalar.copy if gi in (0, 3, 4)
                             else nc.vector.tensor_copy)(
                                ptt[:, g0:g0 + glen, 0:1],
                                sp[:, 0:glen, 0:1])
                        elif gi in act_groups:
                            nc.scalar.activation(
                                ptt[:, g0:g0 + glen, 0:qw],
                                sp[:, 0:glen, 0:qw], Exp)
                        else:
                            nc.vector._custom_dve(
                                exp_op,
                                out=ptt[:, g0:g0 + glen, 0:qw],
                                in0=sp[:, 0:glen, 0:qw],
                                s0=EA, s1=EB, imm2=EC)
                    live[i] = ptt

                def emit_pv(i):
                    qc, h = units[i]
                    q0, qw = QOF[qc], QCS[qc]
                    nsl = (qw + 127) // 128
                    ptt = live.pop(i)
                    if h == 0:
                        emit_pv.ao = aop.tile([128, 4, HPC, HD], f16,
                                              tag="ao", name=f"ao{qc}")
                        emit_pv.diag = aop.tile([128, 4, HPC, 128], f16,
                                                tag="diag", name=f"dg{qc}")
                    ao, diag = emit_pv.ao, emit_pv.diag
                    pvu = upool.tile([128, 512], f32, tag="u",
                                     name=f"pv{qc}_{h}")
                    pv = pvu[:, 0:260].rearrange("p (s d) -> p s d", s=4)
                    for s in range(nsl):
                        sw = min(128, qw - s * 128)
                        s0 = s * 128
                        for k in range(KT):
                            nc.tensor.matmul(
                                pv[0:sw, s, :],
                                ptt[:, k, s0:s0 + sw],
                                vp[:, k, h, :],
                                start=(k == 0), stop=(k == KT - 1),
                            )
                    rc = aop.tile([128, 4], f32, tag="rc", name=f"rc{qc}_{h}")
                    nc.vector.reciprocal(rc[:, 0:nsl], pv[:, 0:nsl, HD])
                    # raw (unnormalized) attnout; normalization rides the
                    # transpose matmul via a diag(1/D) moving operand
                    (nc.vector.tensor_copy if h % 2 else nc.scalar.copy)(
                        ao[:, 0:nsl, h, :], pv[:, 0:nsl, 0:HD])
                    for s in range(nsl):
                        sw = min(128, qw - s * 128)
                        if diag_dve:
                            nc.vector.tensor_scalar_mul(
                                diag[0:sw, s, h, 0:sw], ident[0:sw, 0:sw],
                                rc[0:sw, s:s + 1])
                        else:
                            nc.gpsimd.tensor_scalar_mul(
                                diag[0:sw, s, h, 0:sw], ident[0:sw, 0:sw],
                                rc[0:sw, s:s + 1])
                    if h == HPC - 1:
                        pending.append((qc, ao, diag))

                def emit_tail_a(qc, ao, diag, p0):
                    q0, qw = QOF[qc], QCS[qc]
                    nsl = (qw + 127) // 128
                    # normalize + transpose attnout back to [c, q] via PE
                    pn = min(2, nsl - p0)
                    tu = upool.tile([128, 512], f32, tag="u",
                                    name=f"t{qc}_{p0}")
                    tA = tu[:, 0:256].rearrange("p (a b) -> p a b", a=2)
                    tB = tu[0:64, 256:512].rearrange(
                        "p (a b) -> p a b", a=2)
                    for j in range(pn):
                        s = p0 + j
                        sw = min(128, qw - s * 128)
                        for h in range(HPC):
                            dst = (tA[h * 64:(h + 1) * 64, j, 0:sw]
                                   if h < 2 else tB[:, j, 0:sw])
                            nc.tensor.matmul(
                                dst, ao[0:sw, s, h, :],
                                diag[0:sw, s, h, 0:sw],
                                start=True, stop=True)
                    c0 = q0 + p0 * 128
                    cw = min(256, qw - p0 * 128)
                    dstA = aoT1[:, c0:c0 + cw]
                    dstB = aoT2[:, c0:c0 + cw]
                    if cw > 128:
                        dstA = dstA.rearrange("p (a b) -> p a b", a=2)
                        dstB = dstB.rearrange("p (a b) -> p a b", a=2)
                        srcA, srcB = tA[:, 0:2, :], tB[:, 0:2, :]
                    else:
                        dstA = dstA[:, None, :]
                        dstB = dstB[:, None, :]
                        srcA, srcB = tA[:, 0:1, 0:cw], tB[:, 0:1, 0:cw]
                    if (p0 // 2) % 2:
                        nc.vector.tensor_copy(dstA, srcA)
                        nc.scalar.copy(dstB, srcB)
                    else:
                        nc.scalar.copy(dstA, srcA)
                        nc.vector.tensor_copy(dstB, srcB)

                def emit_tail_b(qc, part):
                    q0, qw = QOF[qc], QCS[qc]
                    # partial projection for this q chunk (three 2-m slots)
                    if part == 0:
                        emit_tail_b.stg = aop.tile([128, 6, 512], f16,
                                                   tag="stg", name=f"stg{qc}")
                    stg = emit_tail_b.stg
                    ppt = bigp.tile([128, 2, 512], f32, tag="big2",
                                    name=f"pp{qc}_{part}")
                    for m in range(part * 2, part * 2 + 2):
                        pp = ppt[:, m % 2, :]
                        nc.tensor.matmul(pp[:, 0:qw],
                                         pw1[:, m * 128:(m + 1) * 128],
                                         aoT1[:, q0:q0 + qw],
                                         start=True, stop=False)
                        nc.tensor.matmul(pp[:, 0:qw],
                                         pw2[:, m * 128:(m + 1) * 128],
                                         aoT2[:, q0:q0 + qw],
                                         start=False, stop=True)
                        (nc.vector.tensor_copy if m % 2 else nc.scalar.copy)(
                            stg[:, m, 0:qw], pp[:, 0:qw])
                    if part >= 1:
                        m0 = (part - 1) * 3
                        nc.sync.dma_start(
                            d_po[m0:m0 + 3, :, q0:q0 + qw].rearrange(
                                "m p q -> p m q"),
                            stg[:, m0:m0 + 3, 0:qw])

                # software pipeline: PV lags QK/exp by one unit so the PE
                # never stalls waiting for the exp of its own score tile
                jobs = []
                LAG = int(os.environ.get("ARP_LAG", "2"))
                DRIP = int(os.environ.get("ARP_DRIP", "1"))
                nu = len(units)
                for i in range(nu):
                    emit_qk_exp(i)
                    if relinj == "01":
                        if i == 0:
                            for t in (3, 4):
                                emit_rel_t(t)
                        elif i == 1:
                            for t in (5, 6, 7):
                                emit_rel_t(t)
                    elif relinj == "12":
                        if i == 1:
                            for t in (3, 4):
                                emit_rel_t(t)
                        elif i == 2:
                            for t in (5, 6, 7):
                                emit_rel_t(t)
                    elif relinj == "spread":
                        if 1 <= i <= 5:
                            emit_rel_t(i + 2)
                    if i >= LAG:
                        emit_pv(i - LAG)
                    if pending:
                        qc0, ao0, dg0 = pending.pop(0)
                        nsl0 = (QCS[qc0] + 127) // 128
                        for p0 in range(0, nsl0, 2):
                            jobs.append((emit_tail_a, (qc0, ao0, dg0, p0)))
                        for part in range(3):
                            jobs.append((emit_tail_b, (qc0, part)))
                    for _ in range(DRIP):
                        if jobs:
                            fn, a = jobs.pop(0)
                            fn(*a)
                for i in range(nu - LAG, nu):
                    emit_pv(i)
                while jobs:
                    fn, a = jobs.pop(0)
                    fn(*a)
                while pending:
                    qc0, ao0, dg0 = pending.pop(0)
                    nsl0 = (QCS[qc0] + 127) // 128
                    for p0 in range(0, nsl0, 2):
                        emit_tail_a(qc0, ao0, dg0, p0)
                    for part in range(3):
                        emit_tail_b(qc0, part)
                aotp_cm.__exit__(None, None, None)
                aop_cm.__exit__(None, None, None)
                ptp_cm.__exit__(None, None, None)

    nc.compile()
    return nc


def _get_compiled():
    global _cached
    if _cached is None:
        _cached = _build_bass()
    return _cached


def _prepare_in_maps(x, qkv_w, proj_w, proj_b, rel_pos_h, rel_pos_w, rel_pos_t):
    f16 = np.float16
    x = np.asarray(x, np.float32)
    qkv_w = np.asarray(qkv_w, np.float32)
    proj_w = np.asarray(proj_w, np.float32)

    ii = np.arange(HW_)
    rh = 8.0 * np.asarray(rel_pos_h, np.float32)[ii[:, None] - ii[None, :] + (HW_ - 1)]
    rw = 8.0 * np.asarray(rel_pos_w, np.float32)[ii[:, None] - ii[None, :] + (HW_ - 1)]
    tt = np.arange(T)
    rt = 8.0 * np.asarray(rel_pos_t, np.float32)[tt[:, None] - tt[None, :] + (T - 1)]
    rht = rh.reshape(196, HD).T        # [64, 196]  cols: (q_h i) x (k_h)
    rwt = rw.reshape(196, HD).T        # [64, 196]
    rtt = rt.reshape(64, HD).T         # [64, 64]   cols: (q_t) x (k_t)

    # fused (rel_h | rel_t) tables: per (t, i) a [64, 22] block
    rhtdup = np.zeros((64, 8 * 14 * 22), np.float32)
    for t in range(T):
        for i in range(HW_):
            b0 = (t * HW_ + i) * 22
            rhtdup[:, b0:b0 + 14] = rht[:, i * 14:(i + 1) * 14]
            rhtdup[:, b0 + 14:b0 + 22] = rtt[:, t * 8:(t + 1) * 8]

    # K' onehot rows: 0:14 h, 14:22 t, 22:32 zero pad, 32:46 w
    aug = np.zeros((46, NK), np.float32)
    k = np.arange(N)
    aug[(k // 14) % 14, k] = 1.0
    aug[14 + k // S, k] = 1.0
    aug[32 + k % 14, k] = 1.0

    xt_b = [np.ascontiguousarray(x[b].reshape(N, DIM).T) for b in range(B)]

    in_maps = []
    for c in range(N_CORES):
        b = c // 4
        heads = [3 * (c % 4) + j for j in range(HPC)]
        wcols = []
        for h in heads:
            wcols.append(qkv_w[HD * h:HD * (h + 1), :])               # q
            wcols.append(qkv_w[DIM + HD * h:DIM + HD * (h + 1), :])   # k
        wqk = np.concatenate(wcols, axis=0).T                          # [768, 384]
        vcols = [qkv_w[2 * DIM + HD * h:2 * DIM + HD * (h + 1), :] for h in heads]
        wvv = np.concatenate(vcols, axis=0).T                          # [768, 192]
        pcols = np.concatenate([np.arange(HD * h, HD * (h + 1)) for h in heads])
        pwt = proj_w[:, pcols].T                                       # [192, 768]

        cst = np.zeros((128, CX), np.float32)
        cst[:, C_WT:C_WV] = wqk.reshape(6, 128, 384).transpose(1, 0, 2).reshape(128, -1)
        cst[:, C_WV:C_ID] = wvv.reshape(6, 128, 192).transpose(1, 0, 2).reshape(128, -1)
        cst[:, C_ID:C_RHT] = np.eye(128, dtype=np.float32)
        cst[0:64, C_RHT:C_RW] = rhtdup
        cst[0:64, C_RW:C_AUG] = rwt
        cst[64:110, C_AUG:C_PW1] = aug
        cst[:, C_PW1:C_PW2] = pwt[0:128]
        cst[0:64, C_PW2:CX] = pwt[128:192]

        in_maps.append({
            "xt": xt_b[b].astype(f16),
            "cst": cst.astype(f16),
        })
    return in_maps


def _unshard(results, proj_b, dtype):
    proj_b = np.asarray(proj_b, np.float64)
    out = np.zeros((B, T, S, DIM), dtype)
    for b in range(B):
        acc = results[4 * b]["po"].astype(np.float64)
        for c in range(4 * b + 1, 4 * b + 4):
            acc = acc + results[c]["po"].astype(np.float64)
        pot = acc.reshape(DIM, N)          # [6*128, 1568]
        out[b] = (pot.T + proj_b).reshape(T, S, DIM).astype(dtype)
    return out


def kernel(x, qkv_w, proj_w, proj_b, rel_pos_h, rel_pos_w, rel_pos_t):
    from concourse import bass_utils

    nc = _get_compiled()
    in_maps = _prepare_in_maps(x, qkv_w, proj_w, proj_b,
                               rel_pos_h, rel_pos_w, rel_pos_t)
    res = bass_utils.run_bass_kernel_spmd(nc, in_maps,
                                          core_ids=list(range(N_CORES)))
    kernel._last_results = res.results
    return _unshard(res.results, proj_b, np.asarray(x).dtype)
